# revision 7
# baseline (speedup 1.0000x reference)
"""Trainium2 Bass kernel for a 2-layer LSTM (frames of an MNIST-ish model).

Model: xb [2048, 8192] -> frames [T=64, B, 128] -> LSTM(128->512) ->
LSTM(512->512) -> last hidden -> Linear(512->10).

Sharding: data-parallel over batch (2048 -> 256 per core, 8 cores),
weights replicated.  Everything on-chip lives transposed ([feature,
batch]) so the recurrence needs no transposes; weights are transposed
once at startup via the PE, x is transposed at startup and staged
through a DRAM scratch buffer.  Matmuls run in float32r (full PE rate
at free dim 256).
"""

import os
import sys

import numpy as np

for _p in ("/opt/trn_rl_repo", "/root/.axon_site/_ro/trn_rl_repo"):
    if os.path.isdir(_p) and _p not in sys.path:
        sys.path.insert(0, _p)

import concourse.bass as bass  # noqa: E402
import concourse.mybir as mybir  # noqa: E402
import concourse.tile as tile  # noqa: E402
from concourse import bacc  # noqa: E402
from concourse.bass_utils import run_bass_kernel_spmd  # noqa: E402
from concourse.masks import make_identity  # noqa: E402

F32 = mybir.dt.float32
F32R = mybir.dt.float32r
AF = mybir.ActivationFunctionType

B, L, IN, H, OUT = 2048, 8192, 128, 512, 10
T = L // IN  # 64 timesteps
NCORES = 8
BL = B // NCORES  # 256 batch rows per core
G4 = 4 * H  # 2048 gate rows
NKC = H // 128  # 4 hidden k-chunks
NMC = G4 // 128  # 16 gate m-chunks
NB = BL  # matmul moving free dim

_CACHE = {}


def _build(opts=None):
    _defaults = dict(act_bias=False, wk_bufs=2, st_bufs=2, xt_bufs=3, k_outer=False, interleave=False)
    _defaults.update(opts or {})
    opts = _defaults
    nc = bacc.Bacc()
    xb = nc.declare_dram_parameter("xb", [BL, L], F32, isOutput=False)
    W_ih0 = nc.declare_dram_parameter("W_ih0", [G4, IN], F32, isOutput=False)
    W_hh0 = nc.declare_dram_parameter("W_hh0", [G4, H], F32, isOutput=False)
    b0 = nc.declare_dram_parameter("b0", [G4], F32, isOutput=False)
    W_ih1 = nc.declare_dram_parameter("W_ih1", [G4, H], F32, isOutput=False)
    W_hh1 = nc.declare_dram_parameter("W_hh1", [G4, H], F32, isOutput=False)
    b1 = nc.declare_dram_parameter("b1", [G4], F32, isOutput=False)
    W_out = nc.declare_dram_parameter("W_out", [OUT, H], F32, isOutput=False)
    b_out = nc.declare_dram_parameter("b_out", [OUT], F32, isOutput=False)
    out = nc.declare_dram_parameter("out", [BL, OUT], F32, isOutput=True)

    with tile.TileContext(nc) as tc:
        with (
            tc.tile_pool(name="const", bufs=1) as const,
            tc.tile_pool(name="xt_dram", bufs=1, space="DRAM") as xdp,
        ):
            ident = const.tile([128, 128], F32, tag="ident")
            make_identity(nc, ident)

            b0t = const.tile([128, NMC], F32, tag="b0t")
            nc.sync.dma_start(out=b0t, in_=b0[:].rearrange("(m p) -> p m", p=128))
            b1t = const.tile([128, NMC], F32, tag="b1t")
            nc.sync.dma_start(out=b1t, in_=b1[:].rearrange("(m p) -> p m", p=128))
            bot = const.tile([OUT, 1], F32, tag="bot")
            nc.sync.dma_start(out=bot, in_=b_out[:].rearrange("(p o) -> p o", o=1))

            # Weights, transposed to [k, 4H] tiles (one per 128-wide k-chunk),
            # stored as float32r for full-rate matmuls.
            def load_wT(wd, kdim, name):
                nkc = kdim // 128
                wts = [
                    const.tile([128, G4], F32R, tag=f"{name}_{kc}", name=f"{name}_{kc}")
                    for kc in range(nkc)
                ]
                with (
                    tc.tile_pool(name=f"stg_{name}", bufs=8) as stg,
                    tc.tile_pool(name=f"psg_{name}", bufs=4, space="PSUM") as psg,
                ):
                    for mg in range(NMC // 4):
                        sts = []
                        for j in range(4):
                            mc = mg * 4 + j
                            st = stg.tile([128, kdim], F32, tag=f"st{j}")
                            nc.sync.dma_start(
                                out=st, in_=wd[mc * 128 : (mc + 1) * 128, :]
                            )
                            sts.append(st)
                        for kc in range(nkc):
                            pt = psg.tile([128, 512], F32, tag="pt")
                            for j in range(4):
                                nc.tensor.transpose(
                                    pt[:, j * 128 : (j + 1) * 128],
                                    sts[j][:, kc * 128 : (kc + 1) * 128],
                                    ident,
                                )
                            nc.vector.tensor_copy(
                                wts[kc][:, mg * 512 : (mg + 1) * 512], pt
                            )
                return wts

            WT_ih0 = load_wT(W_ih0, IN, "wih0")[0]
            WT_hh0 = load_wT(W_hh0, H, "whh0")
            WT_ih1 = load_wT(W_ih1, H, "wih1")
            WT_hh1 = load_wT(W_hh1, H, "whh1")

            WT_out = const.tile([128, NKC * OUT], F32R, tag="wout")
            with (
                tc.tile_pool(name="stg_wo", bufs=1) as stg,
                tc.tile_pool(name="psg_wo", bufs=2, space="PSUM") as psg,
            ):
                st = stg.tile([OUT, H], F32, tag="st")
                nc.sync.dma_start(out=st, in_=W_out[:, :])
                for kc in range(NKC):
                    pt = psg.tile([128, OUT], F32, tag="pt")
                    nc.tensor.transpose(
                        pt, st[:, kc * 128 : (kc + 1) * 128], ident[:OUT, :OUT]
                    )
                    nc.vector.tensor_copy(WT_out[:, kc * OUT : (kc + 1) * OUT], pt)

            # x transposed per timestep: XT[t] = frames[t].T as [128, 256],
            # staged via DRAM (all 64 do not fit in SBUF next to the weights).
            xts_d = [xdp.tile([128, NB], F32R, tag=f"xt{t}", name=f"xtd{t}") for t in range(T)]
            with (
                tc.tile_pool(name="xstg", bufs=1) as xstg,
                tc.tile_pool(name="xpsum", bufs=4, space="PSUM") as xps,
                tc.tile_pool(name="xsb", bufs=4) as xsb,
            ):
                xs0 = xstg.tile([128, L], F32, tag="xs0")
                xs1 = xstg.tile([128, L], F32, tag="xs1")
                nc.sync.dma_start(out=xs0, in_=xb[0:128, :])
                nc.sync.dma_start(out=xs1, in_=xb[128:256, :])
                for t in range(T):
                    pt = xps.tile([128, NB], F32, tag="pt")
                    nc.tensor.transpose(
                        pt[:, 0:128], xs0[:, t * 128 : (t + 1) * 128], ident
                    )
                    nc.tensor.transpose(
                        pt[:, 128:256], xs1[:, t * 128 : (t + 1) * 128], ident
                    )
                    sb = xsb.tile([128, NB], F32R, tag="sb")
                    nc.vector.tensor_copy(sb, pt)
                    nc.sync.dma_start(out=xts_d[t][:, :], in_=sb)

            # ---- recurrence ----
            with (
                tc.tile_pool(name="ps0", bufs=2, space="PSUM") as ps0,
                tc.tile_pool(name="ps1", bufs=2, space="PSUM") as ps1,
                tc.tile_pool(name="xtp", bufs=opts["xt_bufs"]) as xtp,
                tc.tile_pool(name="state", bufs=opts["st_bufs"]) as stp,
                tc.tile_pool(name="work", bufs=opts["wk_bufs"]) as wkp,
            ):
                zero = wkp.tile([128, NB], F32, tag="zero")
                nc.vector.memset(zero, 0.0)
                h0, c0, h1, c1 = [], [], [], []
                for p in range(NKC):
                    for (lst, tg, dt) in (
                        (h0, f"h0_{p}", F32R),
                        (c0, f"c0_{p}", F32),
                        (h1, f"h1_{p}", F32R),
                        (c1, f"c1_{p}", F32),
                    ):
                        tl = stp.tile([128, NB], dt, tag=tg, name=tg)
                        if dt == F32R:
                            nc.vector.tensor_copy(tl, zero)
                        else:
                            nc.vector.memset(tl, 0.0)
                        lst.append(tl)

                def lstm_step_gen(lname, pspool, pairs, c_prev, bt, res=None):
                    """One LSTM layer timestep in transposed layout.

                    pairs: accumulation list of (wT_tile, rhs_tile); each gate
                    m-chunk accumulates all pairs into PSUM.  PSUM group tile
                    holds (i|f|o|g) for one 128-wide slice of the hidden dim.
                    Yields once per group so two layers can be emitted
                    interleaved.
                    """
                    h_new, c_new = [], []
                    n = len(pairs)
                    for p in range(NKC):
                        ps = pspool.tile([128, 4 * NB], F32, tag="g")
                        if opts["k_outer"]:
                            for idx, (wt, rhs) in enumerate(pairs):
                                for pos, gate in enumerate((0, 1, 3, 2)):
                                    mc = gate * NKC + p
                                    oap = ps[:, pos * NB : (pos + 1) * NB]
                                    nc.tensor.matmul(
                                        oap,
                                        wt[:, mc * 128 : (mc + 1) * 128],
                                        rhs,
                                        start=(idx == 0),
                                        stop=(idx == n - 1),
                                        skip_group_check=True,
                                    )
                        else:
                            for pos, gate in enumerate((0, 1, 3, 2)):  # i, f, o, g
                                mc = gate * NKC + p
                                oap = ps[:, pos * NB : (pos + 1) * NB]
                                for idx, (wt, rhs) in enumerate(pairs):
                                    nc.tensor.matmul(
                                        oap,
                                        wt[:, mc * 128 : (mc + 1) * 128],
                                        rhs,
                                        start=(idx == 0),
                                        stop=(idx == n - 1),
                                    )
                        zb = wkp.tile([128, 3 * NB], F32, tag=f"zb{lname}")
                        for pos, gate in enumerate((0, 1, 3)):  # i, f, o biases
                            mc = gate * NKC + p
                            nc.vector.tensor_scalar_add(
                                zb[:, pos * NB : (pos + 1) * NB],
                                ps[:, pos * NB : (pos + 1) * NB],
                                bt[:, mc : mc + 1],
                            )
                        sg = wkp.tile([128, 3 * NB], F32, tag=f"sg{lname}")
                        nc.scalar.activation(sg, zb, AF.Sigmoid)
                        tg = wkp.tile([128, NB], F32, tag=f"tg{lname}")
                        mcg = 2 * NKC + p
                        nc.scalar.activation(
                            tg,
                            ps[:, 3 * NB : 4 * NB],
                            AF.Tanh,
                            bias=bt[:, mcg : mcg + 1],
                        )
                        u = wkp.tile([128, NB], F32, tag=f"u{lname}")
                        nc.vector.tensor_mul(u, sg[:, 0:NB], tg)
                        v = wkp.tile([128, NB], F32, tag=f"v{lname}")
                        nc.vector.tensor_mul(v, sg[:, NB : 2 * NB], c_prev[p])
                        cn = stp.tile([128, NB], F32, tag=f"c{lname}_{p}")
                        nc.vector.tensor_add(cn, u, v)
                        th = wkp.tile([128, NB], F32, tag=f"th{lname}")
                        nc.scalar.activation(th, cn, AF.Tanh)
                        hn = stp.tile([128, NB], F32R, tag=f"h{lname}_{p}")
                        nc.vector.tensor_mul(hn, sg[:, 2 * NB : 3 * NB], th)
                        h_new.append(hn)
                        c_new.append(cn)
                        yield
                    if res is not None:
                        res[lname] = (h_new, c_new)
                    return

                def lstm_step(lname, pspool, pairs, c_prev, bt):
                    res = {}
                    for _ in lstm_step_gen(lname, pspool, pairs, c_prev, bt, res):
                        pass
                    return res[lname]

                for t in range(T):
                    xt = xtp.tile([128, NB], F32R, tag="xt")
                    nc.sync.dma_start(out=xt, in_=xts_d[t][:, :])
                    pairs0 = [(WT_ih0, xt)] + [
                        (WT_hh0[kc], h0[kc]) for kc in range(NKC)
                    ]
                    # layer 1: hh first (h1 from previous step is ready early),
                    # then ih on this step's h0 chunks as they drain.
                    pairs1 = [(WT_hh1[kc], h1[kc]) for kc in range(NKC)] + [
                        (WT_ih1[kc], h0[kc]) for kc in range(NKC)
                    ]
                    if opts["interleave"]:
                        res = {}
                        g0 = lstm_step_gen("0", ps0, pairs0, c0, b0t, res)
                        g1 = lstm_step_gen("1", ps1, pairs1, c1, b1t, res)
                        alive = [g0, g1]
                        while alive:
                            for g in list(alive):
                                try:
                                    next(g)
                                except StopIteration:
                                    alive.remove(g)
                        h0, c0 = res["0"]
                        h1, c1 = res["1"]
                    else:
                        h0, c0 = lstm_step("0", ps0, pairs0, c0, b0t)
                        h1, c1 = lstm_step("1", ps1, pairs1, c1, b1t)

                # head: out.T [10, 256] = W_out @ h1T + b_out
                psf = ps0.tile([128, 4 * NB], F32, tag="g")
                for kc in range(NKC):
                    nc.tensor.matmul(
                        psf[:OUT, 0:NB],
                        WT_out[:, kc * OUT : (kc + 1) * OUT],
                        h1[kc],
                        start=(kc == 0),
                        stop=(kc == NKC - 1),
                    )
                fo = wkp.tile([128, NB], F32, tag="fo")
                nc.vector.tensor_scalar_add(
                    fo[:OUT, :], psf[:OUT, 0:NB], bot[:, 0:1]
                )
                nc.gpsimd.dma_start(
                    out=out[:, :].rearrange("b o -> o b"), in_=fo[:OUT, :]
                )

    nc.compile()
    return nc


def kernel(**inputs):
    if "nc" not in _CACHE:
        _CACHE["nc"] = _build()
    nc = _CACHE["nc"]

    xb = np.asarray(inputs["xb"], dtype=np.float32)
    shared = {
        k: np.ascontiguousarray(np.asarray(inputs[k], dtype=np.float32))
        for k in (
            "W_ih0",
            "W_hh0",
            "b0",
            "W_ih1",
            "W_hh1",
            "b1",
            "W_out",
            "b_out",
        )
    }
    in_maps = []
    for i in range(NCORES):
        m = dict(shared)
        m["xb"] = np.ascontiguousarray(xb[i * BL : (i + 1) * BL])
        in_maps.append(m)

    trace = bool(int(os.environ.get("KERNEL_TRACE", "0")))
    res = run_bass_kernel_spmd(nc, in_maps, list(range(NCORES)), trace=trace)
    if trace:
        _CACHE["exec_time_ns"] = res.exec_time_ns
    return np.concatenate(
        [res.results[i]["out"] for i in range(NCORES)], axis=0
    )


# revision 8
# speedup vs baseline: 405.5253x; 405.5253x over previous
"""Trainium2 Bass kernel for a 2-layer LSTM (frames of an MNIST-ish model).

Model: xb [2048, 8192] -> frames [T=64, B, 128] -> LSTM(128->512) ->
LSTM(512->512) -> last hidden -> Linear(512->10).

Sharding: data-parallel over batch (2048 -> 256 per core, 8 cores),
weights replicated.  Everything on-chip lives transposed ([feature,
batch]) so the recurrence needs no transposes; weights are transposed
once at startup via the PE, x is transposed at startup and staged
through a DRAM scratch buffer.  Matmuls run in float32r (full PE rate
at free dim 256).
"""

import os
import sys

import numpy as np

for _p in ("/opt/trn_rl_repo", "/root/.axon_site/_ro/trn_rl_repo"):
    if os.path.isdir(_p) and _p not in sys.path:
        sys.path.insert(0, _p)

import concourse.bass as bass  # noqa: E402
import concourse.mybir as mybir  # noqa: E402
import concourse.tile as tile  # noqa: E402
from concourse import bacc  # noqa: E402
from concourse.bass_utils import run_bass_kernel_spmd  # noqa: E402
from concourse.masks import make_identity  # noqa: E402

F32 = mybir.dt.float32
F32R = mybir.dt.float32r
AF = mybir.ActivationFunctionType

B, L, IN, H, OUT = 2048, 8192, 128, 512, 10
T = L // IN  # 64 timesteps
NCORES = 8
BL = B // NCORES  # 256 batch rows per core
G4 = 4 * H  # 2048 gate rows
NKC = H // 128  # 4 hidden k-chunks
NMC = G4 // 128  # 16 gate m-chunks
NB = BL  # matmul moving free dim

_CACHE = {}


def _build(opts=None):
    _defaults = dict(act_bias=False, wk_bufs=2, st_bufs=2, xt_bufs=3, k_outer=False, interleave=False, xt_sbuf=6)
    _defaults.update(opts or {})
    opts = _defaults
    nc = bacc.Bacc()
    xb = nc.declare_dram_parameter("xb", [BL, L], F32, isOutput=False)
    W_ih0 = nc.declare_dram_parameter("W_ih0", [G4, IN], F32, isOutput=False)
    W_hh0 = nc.declare_dram_parameter("W_hh0", [G4, H], F32, isOutput=False)
    b0 = nc.declare_dram_parameter("b0", [G4], F32, isOutput=False)
    W_ih1 = nc.declare_dram_parameter("W_ih1", [G4, H], F32, isOutput=False)
    W_hh1 = nc.declare_dram_parameter("W_hh1", [G4, H], F32, isOutput=False)
    b1 = nc.declare_dram_parameter("b1", [G4], F32, isOutput=False)
    W_out = nc.declare_dram_parameter("W_out", [OUT, H], F32, isOutput=False)
    b_out = nc.declare_dram_parameter("b_out", [OUT], F32, isOutput=False)
    out = nc.declare_dram_parameter("out", [BL, OUT], F32, isOutput=True)

    with tile.TileContext(nc) as tc:
        with (
            tc.tile_pool(name="const", bufs=1) as const,
            tc.tile_pool(name="xt_dram", bufs=1, space="DRAM") as xdp,
        ):
            ident = const.tile([128, 128], F32, tag="ident")
            make_identity(nc, ident)

            b0t = const.tile([128, NMC], F32, tag="b0t")
            nc.sync.dma_start(out=b0t, in_=b0[:].rearrange("(m p) -> p m", p=128))
            b1t = const.tile([128, NMC], F32, tag="b1t")
            nc.sync.dma_start(out=b1t, in_=b1[:].rearrange("(m p) -> p m", p=128))
            bot = const.tile([OUT, 1], F32, tag="bot")
            nc.sync.dma_start(out=bot, in_=b_out[:].rearrange("(p o) -> p o", o=1))

            # Weights, transposed to [k, 4H] tiles (one per 128-wide k-chunk),
            # stored as float32r for full-rate matmuls.
            def load_wT(wd, kdim, name):
                nkc = kdim // 128
                wts = [
                    const.tile([128, G4], F32R, tag=f"{name}_{kc}", name=f"{name}_{kc}")
                    for kc in range(nkc)
                ]
                with (
                    tc.tile_pool(name=f"stg_{name}", bufs=8) as stg,
                    tc.tile_pool(name=f"psg_{name}", bufs=4, space="PSUM") as psg,
                ):
                    for mg in range(NMC // 4):
                        sts = []
                        for j in range(4):
                            mc = mg * 4 + j
                            st = stg.tile([128, kdim], F32, tag=f"st{j}")
                            nc.sync.dma_start(
                                out=st, in_=wd[mc * 128 : (mc + 1) * 128, :]
                            )
                            sts.append(st)
                        for kc in range(nkc):
                            pt = psg.tile([128, 512], F32, tag="pt")
                            for j in range(4):
                                nc.tensor.transpose(
                                    pt[:, j * 128 : (j + 1) * 128],
                                    sts[j][:, kc * 128 : (kc + 1) * 128],
                                    ident,
                                )
                            nc.vector.tensor_copy(
                                wts[kc][:, mg * 512 : (mg + 1) * 512], pt
                            )
                return wts

            WT_ih0 = load_wT(W_ih0, IN, "wih0")[0]
            WT_hh0 = load_wT(W_hh0, H, "whh0")
            WT_ih1 = load_wT(W_ih1, H, "wih1")
            WT_hh1 = load_wT(W_hh1, H, "whh1")

            WT_out = const.tile([128, NKC * OUT], F32R, tag="wout")
            with (
                tc.tile_pool(name="stg_wo", bufs=1) as stg,
                tc.tile_pool(name="psg_wo", bufs=2, space="PSUM") as psg,
            ):
                st = stg.tile([OUT, H], F32, tag="st")
                nc.sync.dma_start(out=st, in_=W_out[:, :])
                for kc in range(NKC):
                    pt = psg.tile([128, OUT], F32, tag="pt")
                    nc.tensor.transpose(
                        pt, st[:, kc * 128 : (kc + 1) * 128], ident[:OUT, :OUT]
                    )
                    nc.vector.tensor_copy(WT_out[:, kc * OUT : (kc + 1) * OUT], pt)

            # x transposed per timestep: XT[t] = frames[t].T as [128, 256].
            # First SKIP_DRAM tiles stay in SBUF so the recurrence can start
            # early; the rest round-trip through DRAM scratch (they do not
            # fit in SBUF next to the weights).  xb is loaded in column
            # chunks so the first transposes start after ~1/8 of the load.
            SKIP = opts["xt_sbuf"]
            xts_d = [
                None if t < SKIP else
                xdp.tile([128, NB], F32R, tag=f"xt{t}", name=f"xtd{t}")
                for t in range(T)
            ]
            xts_sb = [
                const.tile([128, NB], F32R, tag=f"xts{t}", name=f"xts{t}")
                if t < SKIP else None
                for t in range(T)
            ]
            with (
                tc.tile_pool(name="xstg", bufs=1) as xstg,
                tc.tile_pool(name="xpsum", bufs=4, space="PSUM") as xps,
                tc.tile_pool(name="xsb", bufs=4) as xsb,
            ):
                xs0 = xstg.tile([128, L], F32, tag="xs0")
                xs1 = xstg.tile([128, L], F32, tag="xs1")
                CHUNK_T = 8  # timesteps per load chunk
                for tch in range(0, T, CHUNK_T):
                    lo, hi = tch * IN, (tch + CHUNK_T) * IN
                    nc.sync.dma_start(out=xs0[:, lo:hi], in_=xb[0:128, lo:hi])
                    nc.sync.dma_start(out=xs1[:, lo:hi], in_=xb[128:256, lo:hi])
                for t in range(T):
                    pt = xps.tile([128, NB], F32, tag="pt")
                    nc.tensor.transpose(
                        pt[:, 0:128], xs0[:, t * 128 : (t + 1) * 128], ident
                    )
                    nc.tensor.transpose(
                        pt[:, 128:256], xs1[:, t * 128 : (t + 1) * 128], ident
                    )
                    if t < SKIP:
                        nc.vector.tensor_copy(xts_sb[t], pt)
                    else:
                        sb = xsb.tile([128, NB], F32R, tag="sb")
                        nc.vector.tensor_copy(sb, pt)
                        nc.sync.dma_start(out=xts_d[t][:, :], in_=sb)

            # ---- recurrence ----
            with (
                tc.tile_pool(name="ps0", bufs=2, space="PSUM") as ps0,
                tc.tile_pool(name="ps1", bufs=2, space="PSUM") as ps1,
                tc.tile_pool(name="xtp", bufs=opts["xt_bufs"]) as xtp,
                tc.tile_pool(name="state", bufs=opts["st_bufs"]) as stp,
                tc.tile_pool(name="work", bufs=opts["wk_bufs"]) as wkp,
            ):
                zero = wkp.tile([128, NB], F32, tag="zero")
                nc.vector.memset(zero, 0.0)
                h0, c0, h1, c1 = [], [], [], []
                for p in range(NKC):
                    for (lst, tg, dt) in (
                        (h0, f"h0_{p}", F32R),
                        (c0, f"c0_{p}", F32),
                        (h1, f"h1_{p}", F32R),
                        (c1, f"c1_{p}", F32),
                    ):
                        tl = stp.tile([128, NB], dt, tag=tg, name=tg)
                        if dt == F32R:
                            nc.vector.tensor_copy(tl, zero)
                        else:
                            nc.vector.memset(tl, 0.0)
                        lst.append(tl)

                def lstm_step_gen(lname, pspool, pairs, c_prev, bt, res=None):
                    """One LSTM layer timestep in transposed layout.

                    pairs: accumulation list of (wT_tile, rhs_tile); each gate
                    m-chunk accumulates all pairs into PSUM.  PSUM group tile
                    holds (i|f|o|g) for one 128-wide slice of the hidden dim.
                    Yields once per group so two layers can be emitted
                    interleaved.
                    """
                    h_new, c_new = [], []
                    n = len(pairs)
                    for p in range(NKC):
                        ps = pspool.tile([128, 4 * NB], F32, tag="g")
                        if opts["k_outer"]:
                            for idx, (wt, rhs) in enumerate(pairs):
                                for pos, gate in enumerate((0, 1, 3, 2)):
                                    mc = gate * NKC + p
                                    oap = ps[:, pos * NB : (pos + 1) * NB]
                                    nc.tensor.matmul(
                                        oap,
                                        wt[:, mc * 128 : (mc + 1) * 128],
                                        rhs,
                                        start=(idx == 0),
                                        stop=(idx == n - 1),
                                        skip_group_check=True,
                                    )
                        else:
                            for pos, gate in enumerate((0, 1, 3, 2)):  # i, f, o, g
                                mc = gate * NKC + p
                                oap = ps[:, pos * NB : (pos + 1) * NB]
                                for idx, (wt, rhs) in enumerate(pairs):
                                    nc.tensor.matmul(
                                        oap,
                                        wt[:, mc * 128 : (mc + 1) * 128],
                                        rhs,
                                        start=(idx == 0),
                                        stop=(idx == n - 1),
                                    )
                        zb = wkp.tile([128, 3 * NB], F32, tag=f"zb{lname}")
                        for pos, gate in enumerate((0, 1, 3)):  # i, f, o biases
                            mc = gate * NKC + p
                            nc.vector.tensor_scalar_add(
                                zb[:, pos * NB : (pos + 1) * NB],
                                ps[:, pos * NB : (pos + 1) * NB],
                                bt[:, mc : mc + 1],
                            )
                        sg = wkp.tile([128, 3 * NB], F32, tag=f"sg{lname}")
                        nc.scalar.activation(sg, zb, AF.Sigmoid)
                        tg = wkp.tile([128, NB], F32, tag=f"tg{lname}")
                        mcg = 2 * NKC + p
                        nc.scalar.activation(
                            tg,
                            ps[:, 3 * NB : 4 * NB],
                            AF.Tanh,
                            bias=bt[:, mcg : mcg + 1],
                        )
                        u = wkp.tile([128, NB], F32, tag=f"u{lname}")
                        nc.vector.tensor_mul(u, sg[:, 0:NB], tg)
                        v = wkp.tile([128, NB], F32, tag=f"v{lname}")
                        nc.vector.tensor_mul(v, sg[:, NB : 2 * NB], c_prev[p])
                        cn = stp.tile([128, NB], F32, tag=f"c{lname}_{p}")
                        nc.vector.tensor_add(cn, u, v)
                        th = wkp.tile([128, NB], F32, tag=f"th{lname}")
                        nc.scalar.activation(th, cn, AF.Tanh)
                        hn = stp.tile([128, NB], F32R, tag=f"h{lname}_{p}")
                        nc.vector.tensor_mul(hn, sg[:, 2 * NB : 3 * NB], th)
                        h_new.append(hn)
                        c_new.append(cn)
                        yield
                    if res is not None:
                        res[lname] = (h_new, c_new)
                    return

                def lstm_step(lname, pspool, pairs, c_prev, bt):
                    res = {}
                    for _ in lstm_step_gen(lname, pspool, pairs, c_prev, bt, res):
                        pass
                    return res[lname]

                for t in range(T):
                    if xts_sb[t] is not None:
                        xt = xts_sb[t]
                    else:
                        xt = xtp.tile([128, NB], F32R, tag="xt")
                        nc.sync.dma_start(out=xt, in_=xts_d[t][:, :])
                    pairs0 = [(WT_ih0, xt)] + [
                        (WT_hh0[kc], h0[kc]) for kc in range(NKC)
                    ]
                    # layer 1: hh first (h1 from previous step is ready early),
                    # then ih on this step's h0 chunks as they drain.
                    pairs1 = [(WT_hh1[kc], h1[kc]) for kc in range(NKC)] + [
                        (WT_ih1[kc], h0[kc]) for kc in range(NKC)
                    ]
                    if opts["interleave"]:
                        res = {}
                        g0 = lstm_step_gen("0", ps0, pairs0, c0, b0t, res)
                        g1 = lstm_step_gen("1", ps1, pairs1, c1, b1t, res)
                        alive = [g0, g1]
                        while alive:
                            for g in list(alive):
                                try:
                                    next(g)
                                except StopIteration:
                                    alive.remove(g)
                        h0, c0 = res["0"]
                        h1, c1 = res["1"]
                    else:
                        h0, c0 = lstm_step("0", ps0, pairs0, c0, b0t)
                        h1, c1 = lstm_step("1", ps1, pairs1, c1, b1t)

                # head: out.T [10, 256] = W_out @ h1T + b_out
                psf = ps0.tile([128, 4 * NB], F32, tag="g")
                for kc in range(NKC):
                    nc.tensor.matmul(
                        psf[:OUT, 0:NB],
                        WT_out[:, kc * OUT : (kc + 1) * OUT],
                        h1[kc],
                        start=(kc == 0),
                        stop=(kc == NKC - 1),
                    )
                fo = wkp.tile([128, NB], F32, tag="fo")
                nc.vector.tensor_scalar_add(
                    fo[:OUT, :], psf[:OUT, 0:NB], bot[:, 0:1]
                )
                nc.gpsimd.dma_start(
                    out=out[:, :].rearrange("b o -> o b"), in_=fo[:OUT, :]
                )

    nc.compile()
    return nc


def kernel(**inputs):
    if "nc" not in _CACHE:
        _CACHE["nc"] = _build()
    nc = _CACHE["nc"]

    xb = np.asarray(inputs["xb"], dtype=np.float32)
    shared = {
        k: np.ascontiguousarray(np.asarray(inputs[k], dtype=np.float32))
        for k in (
            "W_ih0",
            "W_hh0",
            "b0",
            "W_ih1",
            "W_hh1",
            "b1",
            "W_out",
            "b_out",
        )
    }
    in_maps = []
    for i in range(NCORES):
        m = dict(shared)
        m["xb"] = np.ascontiguousarray(xb[i * BL : (i + 1) * BL])
        in_maps.append(m)

    trace = bool(int(os.environ.get("KERNEL_TRACE", "0")))
    res = run_bass_kernel_spmd(nc, in_maps, list(range(NCORES)), trace=trace)
    if trace:
        _CACHE["exec_time_ns"] = res.exec_time_ns
    return np.concatenate(
        [res.results[i]["out"] for i in range(NCORES)], axis=0
    )
